# revision 9
# baseline (speedup 1.0000x reference)
"""CenterLoss update kernel for Trainium2, 8-core SPMD.

Reference computation (N=16384 samples, C=10000 classes, D=128 dims):
    embeded_labels = labels @ center          # [N,D] gather via one-hot
    diff = embeded_labels - embeded_preds
    grad = (labels.T @ diff) / (counts + 1)   # counts = labels.T @ ones
    out  = center - 0.5 * grad

Because each row of ``labels`` is one-hot, ``labels.T @ labels == diag(counts)``,
so the whole thing collapses to a single pass over ``labels``:

    S      = labels.T @ embeded_preds         # [C,D] per-class sum of preds
    counts = column sums of labels            # [C]
    out    = beta * center + gamma * S
             beta  = 1 - 0.5*counts/(counts+1)
             gamma = 0.5/(counts+1)

Sharding: by CLASS, not batch.  Core i owns classes [i*1250, (i+1)*1250): it
streams labels[:, shard] (the same 82 MB/core the batch split would read),
additionally reads all of preds (only 8.4 MB), and produces its [1250, 128]
slice of the output with NO inter-core communication at all -- the batch-split
variant needs a serial ReduceScatter of the [C,D] partials which measured
~85us of un-overlappable tail.

The 655MB ``labels`` tensor is the whole game (target ~358 GB/s/core HBM):
  * it streams through the PE exactly once as the *moving* matmul operand in
    a single fp32r pass (fp32r moving rows cost 1 PE cycle at width>=256,
    same as bf16; the rounding error lands ~2e-4 rel, far inside the 2e-2
    gate);
  * each SBUF partition line packs FOUR consecutive DRAM rows, so every DMA
    descriptor is a 20 KB contiguous read (a [128 rows-on-partitions] layout
    gives 5 KB descriptors, which caps each of the 16 SDMA engines at
    ~21 GB/s -- measured); the four row-parities just become four stationary
    preds tiles per k-group;
  * the class shard is host-padded 1250 -> 1280 so all matmul chunks are
    512/512/256 wide -- fp32r moving runs 4 cycles/row below width 256;
  * label DMAs alternate between the two HWDGE rings (sync + scalar) so
    ring issue costs overlap across consecutive tiles.
Per-partition partial counts accumulate on the DVE and are reduced by one
final PE pass against a ones vector; the [d, cs] S.T accumulator stays in
PSUM for the whole stream and is transposed on-chip for the tail update.
"""

import numpy as np

N, C, D = 16384, 10000, 128
NCORES = 8
CS = C // NCORES   # 1250 classes per core
CSP = 1280         # padded shard width (host pads with zero columns)
LR = 0.5
P = 128
R = 4              # DRAM rows interleaved per SBUF partition line
GR = R * P         # rows per k-group (512)
PJ = 8             # preds load chunks (at full size)


def _chunks(width, step=512):
    out = []
    c0 = 0
    while c0 < width:
        out.append((c0, min(step, width - c0)))
        c0 += step
    return out


def build_program(n=N, cs=CS, csp=CSP, d=D):
    """Build the SPMD Bass program (identical on every core)."""
    import concourse.bacc as bacc
    import concourse.mybir as mybir
    import concourse.tile as tile
    from concourse.masks import make_identity

    f32 = mybir.dt.float32
    f32r = mybir.dt.float32r
    mult = mybir.AluOpType.mult
    add = mybir.AluOpType.add

    assert n % GR == 0
    ng = n // GR             # k-groups (32)
    pj = min(PJ, ng)         # preds chunks
    pgc = ng // pj           # k-groups per preds chunk
    assert ng % pj == 0
    nt3 = (cs + P - 1) // P  # class tiles for the final update (10)

    nc = bacc.Bacc(
        "TRN2",
        target_bir_lowering=False,
        debug=False,
        num_devices=NCORES,
    )

    preds = nc.dram_tensor("preds", [n, d], f32, kind="ExternalInput").ap()
    # labels are one-hot 0/1: declaring them float32r (same bits, trivially
    # rounded) lets plain HWDGE DMAs feed fp32r matmuls at full speed.
    labels = nc.dram_tensor("labels", [n, csp], f32r, kind="ExternalInput").ap()
    center = nc.dram_tensor("center", [cs, d], f32, kind="ExternalInput").ap()
    out = nc.dram_tensor("out", [cs, d], f32, kind="ExternalOutput").ap()

    with tile.TileContext(nc) as tc:
        with tc.tile_pool(name="const", bufs=1) as const_pool:
            identity = const_pool.tile([P, P], f32, name="identity")
            make_identity(nc, identity[:])
            ones_col = const_pool.tile([P, 1], f32, name="ones_col")
            nc.vector.memset(ones_col[:], 1.0)

            # all of preds as ng*R stationary [K=128, M=d] tiles (one per
            # k-group x row-parity), rounded to fp32r by the DVE (a plain
            # dtype-converting copy) so a single fp32r matmul pass suffices.
            preds_r = const_pool.tile([P, ng * R * d], f32r, name="preds_r")

            # per-partition partial counts, accumulated on DVE
            cnt_sb = const_pool.tile([P, csp], f32, name="cnt_sb")
            # S.T evacuated from PSUM, and the reduced counts row
            st_sb = const_pool.tile([P, cs], f32, name="st_sb")
            cnt_row = const_pool.tile([1, cs], f32, name="cnt_row")
            # center shard, as nt3 [class, d] tiles (class on partitions)
            ctr_sb = const_pool.tile([P, nt3 * d], f32, name="ctr_sb")

            # ---------------- phase 1: stream labels ----------------
            with (
                tc.tile_pool(name="lab", bufs=4) as lab_pool,
                tc.tile_pool(name="stage", bufs=2) as stage_pool,
                tc.tile_pool(name="psum1", bufs=1, space="PSUM") as psum1,
            ):
                st_psum = psum1.tile([d, csp], f32, name="st_psum", tag="st",
                                     space="PSUM")

                def load_preds_chunk(j):
                    # one preds chunk (pgc k-groups): HWDGE load of the f32
                    # rows -- partition p takes rows 4p..4p+3 of each k-group
                    # (2 KB contiguous descriptors) -- then a dtype-converting
                    # DVE copy rounds to f32r.  Emission order IS dataflow
                    # order in Tile, so chunk j must be emitted before the
                    # first matmul that reads it.
                    stage = stage_pool.tile([P, pgc * R * d], f32,
                                            name=f"pstage_{j}", tag="stage")
                    nc.scalar.dma_start(
                        out=stage[:].rearrange("p (k r d) -> p k r d",
                                               k=pgc, r=R),
                        in_=preds[j * pgc * GR:(j + 1) * pgc * GR, :]
                            .rearrange("(k p r) d -> p k r d", p=P, r=R),
                    )
                    nc.vector.tensor_copy(
                        out=preds_r[:, j * pgc * R * d:(j + 1) * pgc * R * d],
                        in_=stage[:],
                    )

                load_preds_chunk(0)
                for g in range(ng):
                    lab_g = lab_pool.tile([P, R * csp], f32r, name=f"lab_{g}",
                                          tag="lab")
                    # alternate the two HWDGE rings so ring issue costs and
                    # completion latencies overlap across consecutive tiles;
                    # partition p reads DRAM rows g*512+4p..+3 = one 20 KB
                    # contiguous descriptor.
                    eng = nc.sync if g % 2 == 0 else nc.scalar
                    eng.dma_start(
                        out=lab_g[:],
                        in_=labels[g * GR:(g + 1) * GR, :]
                            .rearrange("(p r) c -> p (r c)", r=R),
                    )
                    for j in range(1, pj):
                        # interleave the remaining preds chunks into the
                        # scalar ring, each well before its first consumer
                        # (chunk j feeds k-groups g >= j*pgc)
                        if g == min(2 * j - 1, j * pgc - 1):
                            load_preds_chunk(j)
                    if g == min(5, ng - 1):
                        # center shard, needed only for the tail update
                        for tt in range(nt3):
                            w = min(P, cs - tt * P)
                            nc.scalar.dma_start(
                                out=ctr_sb[0:w, tt * d:tt * d + d],
                                in_=center[tt * P:tt * P + w, :],
                            )
                    for q in range(R):
                        stat = preds_r[:, (g * R + q) * d:(g * R + q + 1) * d]
                        for c0, w in _chunks(csp):
                            nc.tensor.matmul(
                                out=st_psum[:, c0:c0 + w],
                                lhsT=stat,
                                rhs=lab_g[:, q * csp + c0:q * csp + c0 + w],
                                start=(g == 0 and q == 0),
                                stop=(g == ng - 1 and q == R - 1),
                            )
                        if g == 0 and q == 0:
                            nc.vector.tensor_copy(
                                out=cnt_sb[:],
                                in_=lab_g[:, 0:csp].bitcast(f32),
                            )
                        else:
                            nc.vector.tensor_add(
                                out=cnt_sb[:], in0=cnt_sb[:],
                                in1=lab_g[:, q * csp:(q + 1) * csp].bitcast(f32),
                            )

                # reduce the 128 partial count rows with a ones matmul
                cnt_psum = psum1.tile([1, csp], f32, name="cnt_psum", tag="cnt",
                                      space="PSUM")
                for c0, w in _chunks(csp):
                    nc.tensor.matmul(
                        out=cnt_psum[0:1, c0:c0 + w],
                        lhsT=ones_col[:],
                        rhs=cnt_sb[:, c0:c0 + w],
                        start=True,
                        stop=True,
                    )
                # ACT evacuates both PSUM tiles so phase 3 can reuse the banks
                nc.scalar.copy(out=st_sb[:], in_=st_psum[:, 0:cs])
                nc.scalar.copy(out=cnt_row[:], in_=cnt_psum[0:1, 0:cs])

            # ---------------- phase 2: update this core's shard ----------------
            with (
                tc.tile_pool(name="p3", bufs=2) as p3,
                tc.tile_pool(name="psum3", bufs=2, space="PSUM") as psum3,
            ):
                for tt in range(nt3):
                    w = min(P, cs - tt * P)
                    trp = psum3.tile([P, d], f32, name=f"trp_{tt}", tag="trp",
                                     space="PSUM")
                    nc.tensor.transpose(
                        out=trp[0:w, 0:d],
                        in_=st_sb[:, tt * P:tt * P + w],
                        identity=identity[:, 0:d],
                    )
                    cntc = psum3.tile([P, 1], f32, name=f"cntc_{tt}", tag="cntc",
                                      space="PSUM")
                    nc.tensor.transpose(
                        out=cntc[0:w, 0:1],
                        in_=cnt_row[0:1, tt * P:tt * P + w],
                        identity=identity[0:1, 0:1],
                    )
                    den = p3.tile([P, 1], f32, name=f"den_{tt}", tag="den")
                    nc.vector.tensor_scalar_add(
                        out=den[0:w, :], in0=cntc[0:w, :], scalar1=1.0
                    )
                    rec = p3.tile([P, 1], f32, name=f"rec_{tt}", tag="rec")
                    nc.vector.reciprocal(out=rec[0:w, :], in_=den[0:w, :])
                    gam = p3.tile([P, 1], f32, name=f"gam_{tt}", tag="gam")
                    nc.vector.tensor_scalar_mul(
                        out=gam[0:w, :], in0=rec[0:w, :], scalar1=0.5
                    )
                    bet = p3.tile([P, 1], f32, name=f"bet_{tt}", tag="bet")
                    nc.vector.tensor_tensor(
                        out=bet[0:w, :], in0=cntc[0:w, :], in1=rec[0:w, :],
                        op=mult,
                    )
                    nc.vector.tensor_scalar(
                        out=bet[0:w, :], in0=bet[0:w, :],
                        scalar1=-0.5, scalar2=1.0, op0=mult, op1=add,
                    )
                    o1 = p3.tile([P, d], f32, name=f"o1_{tt}", tag="o1")
                    nc.vector.tensor_scalar_mul(
                        out=o1[0:w, :], in0=ctr_sb[0:w, tt * d:tt * d + d],
                        scalar1=bet[0:w, :],
                    )
                    ou = p3.tile([P, d], f32, name=f"ou_{tt}", tag="ou")
                    nc.vector.scalar_tensor_tensor(
                        out=ou[0:w, :], in0=trp[0:w, 0:d], scalar=gam[0:w, :],
                        in1=o1[0:w, :], op0=mult, op1=add,
                    )
                    nc.sync.dma_start(
                        out=out[tt * P:tt * P + w, :], in_=ou[0:w, 0:d]
                    )

    nc.compile()
    return nc


_PROGRAM = None
LAST_RESULTS = None  # BassKernelResults from the most recent run (for test.py)


def _get_program():
    global _PROGRAM
    if _PROGRAM is None:
        _PROGRAM = build_program()
    return _PROGRAM


def kernel(embeded_preds, labels, center):
    from concourse.bass_utils import run_bass_kernel_spmd

    global LAST_RESULTS
    preds = np.ascontiguousarray(np.asarray(embeded_preds, dtype=np.float32))
    lab = np.asarray(labels, dtype=np.float32)
    ctr = np.ascontiguousarray(np.asarray(center, dtype=np.float32))
    assert preds.shape == (N, D) and lab.shape == (N, C) and ctr.shape == (C, D)

    nc = _get_program()
    in_maps = []
    for i in range(NCORES):
        shard = np.zeros((N, CSP), np.float32)
        shard[:, :CS] = lab[:, i * CS:(i + 1) * CS]
        in_maps.append({
            "preds": preds,
            "labels": shard,
            "center": np.ascontiguousarray(ctr[i * CS:(i + 1) * CS]),
        })
    res = run_bass_kernel_spmd(nc, in_maps, core_ids=list(range(NCORES)))
    LAST_RESULTS = res
    return np.concatenate([res.results[i]["out"] for i in range(NCORES)], axis=0)
